# revision 1
# baseline (speedup 1.0000x reference)
"""Trainium2 Bass kernel for nn_CROM_Layer_81140522156285 (moe_routing).

Math restructure (exactly equivalent to the reference, far less work):
  last = x[:, -1, :]
  q    = last @ Wq.T
  qk   = (q @ Wk) / sqrt(D)              # tiny [B, D]
  scores[b, s] = x[b, s, :] . qk[b, :]   # one pass over x  (big, memory-bound)
  attn = softmax(scores)                 # = exp(s) / sum(exp(s)), s is O(1)
  ctx  = (attn[b] @ x[b]) @ Wv.T         # second contraction folded into same pass
  out  = ctx @ expert_W[eid].T + expert_b[eid]
  y    = x with last row replaced by LayerNorm(last + out)

The only work proportional to x (128 MiB) is scores + the attn-weighted sum
of x rows.  Both are fused into a single pass on device: each [128, D] tile
contributes scores via a fused DVE multiply+reduce, exp via ScalarE, and the
unnormalized weighted row-sum + partition-sum-of-weights via TensorE matmuls
accumulated in PSUM.  Sharding: sequence dim S=8192 split 1024-per-core
across 8 cores (softmax partials combine linearly).  Each core returns
[B, D] unnormalized context + [B, 1] partition function; the host combines
(tiny) and applies the remaining [B, D]-sized projections / LayerNorm.
"""

import numpy as np

import concourse.bass as bass
import concourse.tile as tile
from concourse.bass import _add_dep_helper
from concourse import bacc, mybir
from concourse.bass_utils import run_bass_kernel_spmd

B = 4
S = 8192
D = 1024
N_CORES = 8
S_CORE = S // N_CORES      # positions per batch handled by one core
P = 128                    # SBUF partitions
NT = S_CORE // P           # s-tiles of 128 positions per batch per core
CHUNK_NT = 4               # s-tiles per DMA (2 MiB per dma_start)
NCHUNK = NT // CHUNK_NT

_NC = None


def _build_nc():
    nc = bacc.Bacc("TRN2", target_bir_lowering=False, debug=False,
                   num_devices=N_CORES)
    f32 = mybir.dt.float32
    f32r = mybir.dt.float32r
    x_ap = nc.dram_tensor("x", [B, S_CORE, D], f32r, kind="ExternalInput").ap()
    ones_ap = nc.dram_tensor("ones", [P, 2], f32r, kind="ExternalInput").ap()
    qkb_ap = nc.dram_tensor("qkb", [P, B, D], f32, kind="ExternalInput").ap()
    ctx_ap = nc.dram_tensor("ctx_out", [B, D + 2], f32,
                            kind="ExternalOutput").ap()

    with tile.TileContext(nc) as tc:
        with (
            tc.tile_pool(name="const", bufs=1) as const_pool,
            tc.tile_pool(name="x", bufs=5) as xpool,
            tc.tile_pool(name="scr", bufs=4) as scrpool,
            tc.tile_pool(name="sc", bufs=8) as scpool,
            tc.tile_pool(name="psum", bufs=2, space="PSUM") as psumpool,
            tc.tile_pool(name="stg", bufs=2) as stgpool,
        ):
            ones = const_pool.tile([P, 2], f32r, tag="ones")
            nc.scalar.dma_start(ones[:], ones_ap[:])
            qkts = []
            for qb in range(B):
                t = const_pool.tile([P, D], f32, tag=f"qk{qb}")
                qkts.append(t)
                # all qk pieces on the scalar ring: qk0 streams in parallel
                # with x chunk 0 on the sync ring
                nc.scalar.dma_start(t[:], qkb_ap[:, qb, :])

            # deferred per-batch epilogue: PSUM->SBUF staging + output DMAs,
            # emitted after the NEXT batch's compute so the in-order engine
            # streams don't stall on the stop-matmul at batch transitions
            pending = []

            def flush_pending():
                for ps_c0_, ps_c1_, ps_z_, b_ in pending:
                    stg = stgpool.tile([1, D + 2], f32, tag="stg")
                    nc.vector.tensor_copy(stg[:, 0:512], ps_c0_[:])
                    nc.vector.tensor_copy(stg[:, 512:1024], ps_c1_[:])
                    nc.vector.tensor_copy(stg[:, 1024:1026], ps_z_[:])
                    nc.scalar.dma_start(ctx_ap[b_:b_ + 1, :], stg[:])
                pending.clear()

            x_dmas = []
            for b in range(B):
                # [P, NT, D] view: (p, n, d) -> x[b, p*NT + n, d]; per
                # partition a chunk of CHUNK_NT rows is HBM-contiguous
                xb = x_ap[b, :, :].rearrange("(p n) d -> p n d", p=P)
                ps_c0 = psumpool.tile([1, 512], f32, tag="c0")
                ps_c1 = psumpool.tile([1, 512], f32, tag="c1")
                ps_z = psumpool.tile([1, 2], f32, tag="z")
                widths = [1, 1, 2, 4] if b == 0 else [CHUNK_NT] * NCHUNK
                off = 0
                for ci, w in enumerate(widths):
                    xt = xpool.tile([P, CHUNK_NT, D], f32r, tag="xt")
                    dma_i = nc.sync.dma_start(
                        xt[:, 0:w, :], xb[:, off:off + w, :])
                    x_dmas.append(dma_i.ins)
                    for j in range(w):
                        n = off + j
                        prod = scrpool.tile([P, D], f32, tag="prod")
                        nc.vector.tensor_tensor(
                            out=prod[:], in0=xt[:, j, :], in1=qkts[b][:],
                            op=mybir.AluOpType.mult)
                        sc = scpool.tile([P, 1], f32, tag="sc")
                        dump = scrpool.tile([P, D], f32, tag="dump")
                        nc.scalar.activation(
                            dump[:], prod[:],
                            mybir.ActivationFunctionType.Copy,
                            accum_out=sc[:])
                        esc = scpool.tile([P, 1], f32r, tag="esc")
                        nc.scalar.activation(
                            esc[:], sc[:], mybir.ActivationFunctionType.Exp)
                        st, sp = (n == 0), (n == NT - 1)
                        nc.tensor.matmul(ps_c0[:], esc[:], xt[:, j, 0:512],
                                         start=st, stop=sp)
                        nc.tensor.matmul(ps_c1[:], esc[:], xt[:, j, 512:1024],
                                         start=st, stop=sp)
                        nc.tensor.matmul(ps_z[:], esc[:], ones[:],
                                         start=st, stop=sp)
                        if ci == 0 and j == w - 1:
                            flush_pending()
                    off += w
                pending.append((ps_c0, ps_c1, ps_z, b))
            flush_pending()

    nc.compile()
    return nc


def _get_nc():
    global _NC
    if _NC is None:
        _NC = _build_nc()
    return _NC


def kernel(x_emb, Wq, Wk, Wv, expert_W, expert_b, ln_gamma, ln_beta,
           expert_id, _spmd_kwargs=None):
    x = np.ascontiguousarray(np.asarray(x_emb, dtype=np.float32))
    Wq = np.asarray(Wq, dtype=np.float32)
    Wk = np.asarray(Wk, dtype=np.float32)
    Wv = np.asarray(Wv, dtype=np.float32)
    expert_b = np.asarray(expert_b, dtype=np.float32)
    ln_gamma = np.asarray(ln_gamma, dtype=np.float32)
    ln_beta = np.asarray(ln_beta, dtype=np.float32)
    eid = int(np.asarray(expert_id))

    last = x[:, -1, :]                                   # [B, D]
    q = last @ Wq.T                                      # [B, D]
    qk = (q @ Wk) * np.float32(1.0 / np.sqrt(D))         # [B, D]
    qkb = np.ascontiguousarray(
        np.broadcast_to(qk[None, :, :], (P, B, D)), dtype=np.float32)

    in_maps = [
        {"x": np.ascontiguousarray(x[:, c * S_CORE:(c + 1) * S_CORE, :]),
         "qkb": qkb, "ones": np.ones((P, 2), dtype=np.float32)}
        for c in range(N_CORES)
    ]
    res = run_bass_kernel_spmd(_get_nc(), in_maps, core_ids=list(range(N_CORES)),
                               **(_spmd_kwargs or {}))
    ctx_un = np.zeros((B, D), dtype=np.float32)
    z = np.zeros((B, 1), dtype=np.float32)
    for c in range(N_CORES):
        ctx_un += res.results[c]["ctx_out"][:, 0:D]
        z += res.results[c]["ctx_out"][:, D:D + 1]

    ctx = ctx_un / z                                     # [B, D] attn @ x
    context = ctx @ Wv.T                                 # [B, D]
    We = np.asarray(expert_W[eid], dtype=np.float32)     # [D, D]
    out = context @ We.T + expert_b[eid]                 # [B, D]
    resid = last + out
    mu = resid.mean(axis=-1, keepdims=True, dtype=np.float32)
    diff = resid - mu
    var = np.mean(diff * diff, axis=-1, keepdims=True, dtype=np.float32)
    new_focus = diff / np.sqrt(var + np.float32(1e-5)) * ln_gamma + ln_beta

    y = x.copy()
    y[:, -1, :] = new_focus
    return y


if __name__ == "__main__":
    rng = np.random.default_rng(0)
    xs = {
        "x_emb": rng.standard_normal((B, S, D), dtype=np.float32),
        "Wq": rng.standard_normal((D, D), dtype=np.float32) * 0.02,
        "Wk": rng.standard_normal((D, D), dtype=np.float32) * 0.02,
        "Wv": rng.standard_normal((D, D), dtype=np.float32) * 0.02,
        "expert_W": rng.standard_normal((128, D, D), dtype=np.float32) * 0.02,
        "expert_b": rng.standard_normal((128, D), dtype=np.float32) * 0.02,
        "ln_gamma": np.ones(D, dtype=np.float32),
        "ln_beta": np.zeros(D, dtype=np.float32),
        "expert_id": 7,
    }
    y = kernel(**xs)
    print(y.shape, y.dtype)



# revision 9
# speedup vs baseline: 1.1237x; 1.1237x over previous
"""Trainium2 Bass kernel for nn_CROM_Layer_81140522156285 (moe_routing).

Math restructure (exactly equivalent to the reference, far less work):
  last = x[:, -1, :]
  q    = last @ Wq.T
  qk   = (q @ Wk) / sqrt(D)              # tiny [B, D]
  scores[b, s] = x[b, s, :] . qk[b, :]   # one pass over x  (big, memory-bound)
  attn = softmax(scores)
  ctx  = (attn[b] @ x[b]) @ Wv.T
  out  = ctx @ expert_W[eid].T + expert_b[eid]
  y    = x with last row replaced by LayerNorm(last + out)

Only the scores + attn-weighted row-sum touch all of x.  Device-side design:
  * x is sent in bf16 (halves HBM traffic; tolerance is loose) and packed
    batch-INTERLEAVED: partition p serves batch p//32 for every tile, so one
    [128, D] qk tile works for all tiles and there is a single PSUM
    accumulation group for the whole kernel (no per-batch boundaries).
  * Per [128, D] tile: DVE bf16 2x multiply -> per-partition reduce (split
    between ScalarE copy+accum and GpSimd tensor_reduce to balance engines)
    -> batched exp per chunk on ScalarE -> TensorE matmuls with a [128, 4]
    batch-masked stationary accumulate ctx into PSUM [4, 1024] and z into
    PSUM [32, 1].
  * Sequence dim S=8192 split 1024-per-core across 8 cores (softmax partials
    combine linearly); host combines and applies the tiny tail projections.
"""

import numpy as np
import ml_dtypes

import concourse.bass as bass
import concourse.tile as tile
from concourse import bacc, mybir
from concourse.bass_utils import run_bass_kernel_spmd

B = 4
S = 8192
D = 1024
N_CORES = 8
S_CORE = S // N_CORES      # positions per batch handled by one core
P = 128                    # SBUF partitions
G = P // B                 # partitions per batch group (32)
T = (B * S_CORE) // P      # s-tiles of 128 interleaved positions per core (32)
NCHUNK = 4                 # logical chunks of 8 tiles
W = T // NCHUNK            # tiles per chunk (8)

BF16 = mybir.dt.bfloat16
F32 = mybir.dt.float32

# per-chunk sub-DMA widths (tiles): taper at start for pipeline ramp and at
# the end to shorten the tail
SUB_W = [[1, 1, 2, 4], [4, 4], [4, 4], [4, 2, 2]]
# per-tile score-path engine: 'V' = DVE fused scalar_tensor_tensor
# (mult+accum in one 1x pass), 'M' = GpSimd multiply + ScalarE copy+accum
# reduce, 'S' = DVE 2x multiply + ScalarE copy+accum reduce.  Balanced so
# every engine sits near/below the ~24us DMA floor.
ASSIGN = ["V", "M", "V", "M", "V", "S", "M", "V"]

_NC = None


def _build_nc():
    nc = bacc.Bacc("TRN2", target_bir_lowering=False, debug=False,
                   num_devices=N_CORES)
    xs_ap = nc.dram_tensor("xs", [P, T, D], BF16, kind="ExternalInput").ap()
    # [:, 0:D] = qk row per batch group; [:, D] = 1.0 (z-matmul rhs); [:, D+1] pad
    qkb_ap = nc.dram_tensor("qkb", [P, D + 2], BF16, kind="ExternalInput").ap()
    ctx_ap = nc.dram_tensor("ctx_out", [B, D], F32, kind="ExternalOutput").ap()
    z_ap = nc.dram_tensor("z_out", [T, 1], F32, kind="ExternalOutput").ap()

    with tile.TileContext(nc) as tc:
        with (
            tc.tile_pool(name="const", bufs=1) as cpool,
            tc.tile_pool(name="x", bufs=3) as xpool,
            tc.tile_pool(name="prod", bufs=2) as ppool,
            tc.tile_pool(name="sc", bufs=3) as scpool,
            tc.tile_pool(name="psum", bufs=1, space="PSUM") as psumpool,
            tc.tile_pool(name="stg", bufs=1) as stgpool,
        ):
            qkb = cpool.tile([P, D + 2], BF16, tag="qkb")
            nc.scalar.dma_start(qkb[:], qkb_ap[:])
            qk = qkb[:, 0:D]
            ones = qkb[:, D:D + 1]

            # esc tiles: [P, B, W] bf16, batch-masked (col b nonzero only on
            # partitions of group b).  Pre-zeroed once per buffer; the same
            # masked slots are rewritten every reuse, so zeros stay zero.
            escs = []
            for i in range(min(NCHUNK, 3)):
                e = cpool.tile([P, B, W], BF16, tag=f"esc{i}")
                nc.vector.memset(e[:], 0.0)
                escs.append(e)

            ps_ctx = psumpool.tile([B, D], F32, tag="ctx")      # 2 banks
            ps_z = psumpool.tile([T, 1], F32, tag="z")          # 1 bank

            # per-engine scratch for the full-size elementwise outputs the
            # fused ops are forced to write (never read back)
            dump_v = cpool.tile([P, D], BF16, tag="dump_v")
            dump_a = cpool.tile([P, D], BF16, tag="dump_a")

            n_p = sum(1 for a in ASSIGN if a in ("S", "M"))
            p_idx = {w: j for j, w in
                     enumerate(w for w in range(W) if ASSIGN[w] in ("S", "M"))}
            for ci in range(NCHUNK):
                xt = xpool.tile([P, W, D], BF16, tag="xt")
                prod = ppool.tile([P, n_p, D], BF16, tag="prod")
                sc = scpool.tile([P, W], F32, tag="sc")
                esc = escs[ci % len(escs)]

                off = 0
                for wsub in SUB_W[ci]:
                    xsl = xt[:, off:off + wsub, :]
                    nc.sync.dma_start(xsl, xs_ap[:, ci * W + off:ci * W + off + wsub, :])
                    for w in range(off, off + wsub):
                        if ASSIGN[w] == "V":
                            nc.vector.scalar_tensor_tensor(
                                out=dump_v[:], in0=xt[:, w, :], scalar=1.0,
                                in1=qk, op0=mybir.AluOpType.mult,
                                op1=mybir.AluOpType.mult,
                                accum_out=sc[:, w:w + 1])
                        else:
                            eng = nc.gpsimd if ASSIGN[w] == "M" else nc.vector
                            j = p_idx[w]
                            eng.tensor_tensor(
                                out=prod[:, j, :], in0=xt[:, w, :], in1=qk,
                                op=mybir.AluOpType.mult)
                            nc.scalar.activation(
                                dump_a[:], prod[:, j, :],
                                mybir.ActivationFunctionType.Copy,
                                accum_out=sc[:, w:w + 1])
                    off += wsub

                # batched exp per batch group: sc[g] -> esc[g, b=g, :]
                for g in range(B):
                    nc.scalar.activation(
                        esc[g * G:(g + 1) * G, g:g + 1, :],
                        sc[g * G:(g + 1) * G, :].unsqueeze(1),
                        mybir.ActivationFunctionType.Exp)

                for w in range(W):
                    t = ci * W + w
                    st, sp = (t == 0), (t == T - 1)
                    nc.tensor.matmul(ps_ctx[:, 0:512], esc[:, :, w],
                                     xt[:, w, 0:512], start=st, stop=sp)
                    nc.tensor.matmul(ps_ctx[:, 512:1024], esc[:, :, w],
                                     xt[:, w, 512:1024], start=st, stop=sp)
                nc.tensor.matmul(ps_z[:], esc[:].rearrange("p a b -> p (a b)"),
                                 ones, start=(ci == 0), stop=(ci == NCHUNK - 1))

            stg = stgpool.tile([B, D], F32, tag="stg")
            stgz = stgpool.tile([T, 1], F32, tag="stgz")
            nc.vector.tensor_copy(stgz[:], ps_z[:])
            nc.scalar.dma_start(z_ap[:], stgz[:])
            nc.scalar.activation(stg[:], ps_ctx[:],
                                 mybir.ActivationFunctionType.Copy)
            nc.scalar.dma_start(ctx_ap[:], stg[:])

    nc.compile()
    return nc


def _get_nc():
    global _NC
    if _NC is None:
        _NC = _build_nc()
    return _NC


def kernel(x_emb, Wq, Wk, Wv, expert_W, expert_b, ln_gamma, ln_beta,
           expert_id, _spmd_kwargs=None):
    x = np.ascontiguousarray(np.asarray(x_emb, dtype=np.float32))
    Wq = np.asarray(Wq, dtype=np.float32)
    Wk = np.asarray(Wk, dtype=np.float32)
    Wv = np.asarray(Wv, dtype=np.float32)
    expert_b = np.asarray(expert_b, dtype=np.float32)
    ln_gamma = np.asarray(ln_gamma, dtype=np.float32)
    ln_beta = np.asarray(ln_beta, dtype=np.float32)
    eid = int(np.asarray(expert_id))

    last = x[:, -1, :]                                   # [B, D]
    q = last @ Wq.T                                      # [B, D]
    qk = (q @ Wk) * np.float32(1.0 / np.sqrt(D))         # [B, D]

    # qkb[p, 0:D] = qk[p//G]; [:, D] = 1.0 (ones column); [:, D+1] = 0 pad
    qkb = np.zeros((P, D + 2), dtype=ml_dtypes.bfloat16)
    qkb[:, 0:D] = np.repeat(qk, G, axis=0).astype(ml_dtypes.bfloat16)
    qkb[:, D] = ml_dtypes.bfloat16(1.0)

    # per-core pack: [P, T, D] bf16 with partition p = batch p//G,
    # position within core shard = (p%G)*G + t  -> a single reshape
    in_maps = []
    for c in range(N_CORES):
        shard = x[:, c * S_CORE:(c + 1) * S_CORE, :]     # [B, S_CORE, D]
        xs = np.ascontiguousarray(
            shard.reshape(P, T, D).astype(ml_dtypes.bfloat16))
        in_maps.append({"xs": xs, "qkb": qkb})

    res = run_bass_kernel_spmd(_get_nc(), in_maps, core_ids=list(range(N_CORES)),
                               **(_spmd_kwargs or {}))
    ctx_un = np.zeros((B, D), dtype=np.float32)
    z = np.zeros((B, 1), dtype=np.float32)
    for c in range(N_CORES):
        ctx_un += res.results[c]["ctx_out"]
        z[:, 0] += res.results[c]["z_out"].reshape(B, W).sum(axis=1)

    ctx = ctx_un / z                                     # [B, D] attn @ x
    context = ctx @ Wv.T                                 # [B, D]
    We = np.asarray(expert_W[eid], dtype=np.float32)     # [D, D]
    out = context @ We.T + expert_b[eid]                 # [B, D]
    resid = last + out
    mu = resid.mean(axis=-1, keepdims=True, dtype=np.float32)
    diff = resid - mu
    var = np.mean(diff * diff, axis=-1, keepdims=True, dtype=np.float32)
    new_focus = diff / np.sqrt(var + np.float32(1e-5)) * ln_gamma + ln_beta

    y = x.copy()
    y[:, -1, :] = new_focus
    return y


if __name__ == "__main__":
    rng = np.random.default_rng(0)
    xs = {
        "x_emb": rng.standard_normal((B, S, D), dtype=np.float32),
        "Wq": rng.standard_normal((D, D), dtype=np.float32) * 0.02,
        "Wk": rng.standard_normal((D, D), dtype=np.float32) * 0.02,
        "Wv": rng.standard_normal((D, D), dtype=np.float32) * 0.02,
        "expert_W": rng.standard_normal((128, D, D), dtype=np.float32) * 0.02,
        "expert_b": rng.standard_normal((128, D), dtype=np.float32) * 0.02,
        "ln_gamma": np.ones(D, dtype=np.float32),
        "ln_beta": np.zeros(D, dtype=np.float32),
        "expert_id": 7,
    }
    y = kernel(**xs)
    print(y.shape, y.dtype)


# revision 12
# speedup vs baseline: 1.4050x; 1.2504x over previous
"""Trainium2 Bass kernel for nn_CROM_Layer_81140522156285 (moe_routing).

Math restructure (exactly equivalent to the reference, far less work):
  last = x[:, -1, :]
  q    = last @ Wq.T
  qk   = (q @ Wk) / sqrt(D)              # tiny [B, D]
  scores[b, s] = x[b, s, :] . qk[b, :]   # one pass over x  (big, memory-bound)
  attn = softmax(scores)
  ctx  = (attn[b] @ x[b]) @ Wv.T
  out  = ctx @ expert_W[eid].T + expert_b[eid]
  y    = x with last row replaced by LayerNorm(last + out)

Only the scores + attn-weighted row-sum touch all of x.  Device-side design:
  * x is sent in bf16 (halves HBM traffic; tolerance is loose) and packed
    batch-INTERLEAVED: partition p serves batch p//32 for every tile, so one
    [128, D] qk tile works for all tiles and there is a single PSUM
    accumulation group for the whole kernel (no per-batch boundaries).
  * Per [128, D] tile: DVE bf16 2x multiply -> per-partition reduce (split
    between ScalarE copy+accum and GpSimd tensor_reduce to balance engines)
    -> batched exp per chunk on ScalarE -> TensorE matmuls with a [128, 4]
    batch-masked stationary accumulate ctx into PSUM [4, 1024] and z into
    PSUM [32, 1].
  * Sequence dim S=8192 split 1024-per-core across 8 cores (softmax partials
    combine linearly); host combines and applies the tiny tail projections.
"""

import numpy as np
import ml_dtypes

import concourse.bass as bass
import concourse.tile as tile
from concourse import bacc, mybir
from concourse.bass_utils import run_bass_kernel_spmd

B = 4
S = 8192
D = 1024
N_CORES = 8
S_CORE = S // N_CORES      # positions per batch handled by one core
P = 128                    # SBUF partitions
G = P // B                 # partitions per batch group (32)
T = (B * S_CORE) // P      # s-tiles of 128 interleaved positions per core (32)
NCHUNK = 4                 # logical chunks of 8 tiles
W = T // NCHUNK            # tiles per chunk (8)

BF16 = mybir.dt.bfloat16
F32 = mybir.dt.float32

# per-chunk sub-DMA widths (tiles): taper at start for pipeline ramp and at
# the end to shorten the tail
SUB_W = [[1, 1, 2, 4], [4, 4], [4, 4], [4, 2, 2]]
# per-tile score-path engine: 'V' = DVE fused scalar_tensor_tensor
# (mult+accum in one 1x pass), 'S' = DVE 2x all-bf16 multiply + ScalarE
# copy+accum reduce.  GpSimd is deliberately UNUSED for elementwise work:
# concurrent Pool streaming contends for SBUF ports and measurably halves
# DVE throughput (685ns ops became 2114ns).  Balance DVE vs ACT at ~13/19.
ASSIGN_CHUNK = [
    ["V", "S", "S", "V", "S", "S", "V", "S"],   # 3V 5S
    ["V", "S", "S", "V", "S", "S", "V", "S"],
    ["V", "S", "S", "V", "S", "S", "V", "S"],
    ["V", "S", "V", "S", "V", "S", "V", "S"],   # 4V 4S (13V 19S total)
]

_NC = None


def _build_nc():
    nc = bacc.Bacc("TRN2", target_bir_lowering=False, debug=False,
                   num_devices=N_CORES)
    xs_ap = nc.dram_tensor("xs", [P, T, D], BF16, kind="ExternalInput").ap()
    # [:, 0:D] = qk row per batch group; [:, D] = 1.0 (z-matmul rhs); [:, D+1] pad
    qkb_ap = nc.dram_tensor("qkb", [P, D + 2], BF16, kind="ExternalInput").ap()
    ctx_ap = nc.dram_tensor("ctx_out", [B, D], F32, kind="ExternalOutput").ap()
    z_ap = nc.dram_tensor("z_out", [T, 1], F32, kind="ExternalOutput").ap()

    with tile.TileContext(nc) as tc:
        with (
            tc.tile_pool(name="const", bufs=1) as cpool,
            tc.tile_pool(name="x", bufs=3) as xpool,
            tc.tile_pool(name="prod", bufs=2) as ppool,
            tc.tile_pool(name="sc", bufs=3) as scpool,
            tc.tile_pool(name="psum", bufs=1, space="PSUM") as psumpool,
            tc.tile_pool(name="stg", bufs=1) as stgpool,
        ):
            qkb = cpool.tile([P, D + 2], BF16, tag="qkb")
            nc.scalar.dma_start(qkb[:], qkb_ap[:])
            qk = qkb[:, 0:D]
            ones = qkb[:, D:D + 1]

            # esc tiles: [P, B, W] bf16, batch-masked (col b nonzero only on
            # partitions of group b).  Pre-zeroed once per buffer; the same
            # masked slots are rewritten every reuse, so zeros stay zero.
            escs = []
            for i in range(min(NCHUNK, 3)):
                e = cpool.tile([P, B, W], BF16, tag=f"esc{i}")
                nc.vector.memset(e[:], 0.0)
                escs.append(e)

            ps_ctx = psumpool.tile([B, D], F32, tag="ctx")      # 2 banks
            ps_z = psumpool.tile([T, 1], F32, tag="z")          # 1 bank

            # per-engine scratch for the full-size elementwise outputs the
            # fused ops are forced to write (never read back)
            dump_v = cpool.tile([P, D], BF16, tag="dump_v")
            dump_a = cpool.tile([P, D], BF16, tag="dump_a")

            n_p = max(sum(1 for a in ch if a == "S") for ch in ASSIGN_CHUNK)
            for ci in range(NCHUNK):
                assign = ASSIGN_CHUNK[ci]
                p_idx = {w: j for j, w in
                         enumerate(w for w in range(W) if assign[w] == "S")}
                xt = xpool.tile([P, W, D], BF16, tag="xt")
                prod = ppool.tile([P, n_p, D], BF16, tag="prod")
                sc = scpool.tile([P, W], F32, tag="sc")
                esc = escs[ci % len(escs)]

                off = 0
                for wsub in SUB_W[ci]:
                    xsl = xt[:, off:off + wsub, :]
                    nc.sync.dma_start(xsl, xs_ap[:, ci * W + off:ci * W + off + wsub, :])
                    for w in range(off, off + wsub):
                        if assign[w] == "V":
                            nc.vector.scalar_tensor_tensor(
                                out=dump_v[:], in0=xt[:, w, :], scalar=1.0,
                                in1=qk, op0=mybir.AluOpType.mult,
                                op1=mybir.AluOpType.mult,
                                accum_out=sc[:, w:w + 1])
                        else:
                            j = p_idx[w]
                            nc.vector.tensor_tensor(
                                out=prod[:, j, :], in0=xt[:, w, :], in1=qk,
                                op=mybir.AluOpType.mult)
                            nc.scalar.activation(
                                dump_a[:], prod[:, j, :],
                                mybir.ActivationFunctionType.Copy,
                                accum_out=sc[:, w:w + 1])
                    off += wsub

                # batched exp per batch group: sc[g] -> esc[g, b=g, :]
                for g in range(B):
                    nc.scalar.activation(
                        esc[g * G:(g + 1) * G, g:g + 1, :],
                        sc[g * G:(g + 1) * G, :].unsqueeze(1),
                        mybir.ActivationFunctionType.Exp)

                for w in range(W):
                    t = ci * W + w
                    st, sp = (t == 0), (t == T - 1)
                    nc.tensor.matmul(ps_ctx[:, 0:512], esc[:, :, w],
                                     xt[:, w, 0:512], start=st, stop=sp)
                    nc.tensor.matmul(ps_ctx[:, 512:1024], esc[:, :, w],
                                     xt[:, w, 512:1024], start=st, stop=sp)
                nc.tensor.matmul(ps_z[:], esc[:].rearrange("p a b -> p (a b)"),
                                 ones, start=(ci == 0), stop=(ci == NCHUNK - 1))

            stg = stgpool.tile([B, D], F32, tag="stg")
            stgz = stgpool.tile([T, 1], F32, tag="stgz")
            nc.vector.tensor_copy(stgz[:], ps_z[:])
            nc.scalar.dma_start(z_ap[:], stgz[:])
            nc.scalar.activation(stg[:], ps_ctx[:],
                                 mybir.ActivationFunctionType.Copy)
            nc.scalar.dma_start(ctx_ap[:], stg[:])

    nc.compile()
    return nc


def _get_nc():
    global _NC
    if _NC is None:
        _NC = _build_nc()
    return _NC


def kernel(x_emb, Wq, Wk, Wv, expert_W, expert_b, ln_gamma, ln_beta,
           expert_id, _spmd_kwargs=None):
    x = np.ascontiguousarray(np.asarray(x_emb, dtype=np.float32))
    Wq = np.asarray(Wq, dtype=np.float32)
    Wk = np.asarray(Wk, dtype=np.float32)
    Wv = np.asarray(Wv, dtype=np.float32)
    expert_b = np.asarray(expert_b, dtype=np.float32)
    ln_gamma = np.asarray(ln_gamma, dtype=np.float32)
    ln_beta = np.asarray(ln_beta, dtype=np.float32)
    eid = int(np.asarray(expert_id))

    last = x[:, -1, :]                                   # [B, D]
    q = last @ Wq.T                                      # [B, D]
    qk = (q @ Wk) * np.float32(1.0 / np.sqrt(D))         # [B, D]

    # qkb[p, 0:D] = qk[p//G]; [:, D] = 1.0 (ones column); [:, D+1] = 0 pad
    qkb = np.zeros((P, D + 2), dtype=ml_dtypes.bfloat16)
    qkb[:, 0:D] = np.repeat(qk, G, axis=0).astype(ml_dtypes.bfloat16)
    qkb[:, D] = ml_dtypes.bfloat16(1.0)

    # per-core pack: [P, T, D] bf16 with partition p = batch p//G,
    # position within core shard = (p%G)*G + t  -> a single reshape
    in_maps = []
    for c in range(N_CORES):
        shard = x[:, c * S_CORE:(c + 1) * S_CORE, :]     # [B, S_CORE, D]
        xs = np.ascontiguousarray(
            shard.reshape(P, T, D).astype(ml_dtypes.bfloat16))
        in_maps.append({"xs": xs, "qkb": qkb})

    res = run_bass_kernel_spmd(_get_nc(), in_maps, core_ids=list(range(N_CORES)),
                               **(_spmd_kwargs or {}))
    ctx_un = np.zeros((B, D), dtype=np.float32)
    z = np.zeros((B, 1), dtype=np.float32)
    for c in range(N_CORES):
        ctx_un += res.results[c]["ctx_out"]
        z[:, 0] += res.results[c]["z_out"].reshape(B, W).sum(axis=1)

    ctx = ctx_un / z                                     # [B, D] attn @ x
    context = ctx @ Wv.T                                 # [B, D]
    We = np.asarray(expert_W[eid], dtype=np.float32)     # [D, D]
    out = context @ We.T + expert_b[eid]                 # [B, D]
    resid = last + out
    mu = resid.mean(axis=-1, keepdims=True, dtype=np.float32)
    diff = resid - mu
    var = np.mean(diff * diff, axis=-1, keepdims=True, dtype=np.float32)
    new_focus = diff / np.sqrt(var + np.float32(1e-5)) * ln_gamma + ln_beta

    y = x.copy()
    y[:, -1, :] = new_focus
    return y


if __name__ == "__main__":
    rng = np.random.default_rng(0)
    xs = {
        "x_emb": rng.standard_normal((B, S, D), dtype=np.float32),
        "Wq": rng.standard_normal((D, D), dtype=np.float32) * 0.02,
        "Wk": rng.standard_normal((D, D), dtype=np.float32) * 0.02,
        "Wv": rng.standard_normal((D, D), dtype=np.float32) * 0.02,
        "expert_W": rng.standard_normal((128, D, D), dtype=np.float32) * 0.02,
        "expert_b": rng.standard_normal((128, D), dtype=np.float32) * 0.02,
        "ln_gamma": np.ones(D, dtype=np.float32),
        "ln_beta": np.zeros(D, dtype=np.float32),
        "expert_id": 7,
    }
    y = kernel(**xs)
    print(y.shape, y.dtype)
